# revision 12
# baseline (speedup 1.0000x reference)
"""Trainium2 Bass kernel for nn_Attention_33354716021131.

Dense GQA attention block (B=2, S=2048, D=4096, 32 q-heads / 8 kv-heads,
head_dim 128, RoPE, causal softmax) tensor-parallel across 8 NeuronCores.

Sharding (per core c):
  - heads: q-heads 4c..4c+3 (one kv-head group c) -> wq/wk/wv column shards
  - x^T computed fully locally: every core PE-transposes the whole x into
    SBUF-resident blocks (no x collective at all -> no exposure to the
    collectives entry barrier / launch skew at kernel start)
  - attention entirely local to the core (its 4 q-heads x 2 batches)
  - attention outputs (head-major, transposed) AllGather -> full O^T, then
    wo column shard: core c computes y[:, 512c:512c+512]; host concatenates.

All matmul operands are bfloat16 (fp32 PSUM accumulation): bf16 streams at
1 cycle/row on the PE (fp32/fp32r modes run ~3x slower and do not engage
the HAM clock un-throttle), and halves every DMA/collective payload.
Inputs are cast to bf16 on the host as part of sharding.
"""
import math
import os

import numpy as np

N_CORES = 8
B = 2
S = 2048
DM = 4096
N_HEADS = 32
HD = 128
NQH = N_HEADS // N_CORES          # 4 q heads per core
HDQ = NQH * HD                    # 512
T = B * S                         # 4096 tokens
KC = DM // 128                    # 32 k-chunks
TB = 512                          # token block for projections
NTB = S // TB                     # 4 per batch
QB = 512                          # query block for attention
NQB = S // QB                     # 4
NKT = S // 128                    # 16 key tiles per batch
SCALE = 1.0 / math.sqrt(HD)
ROPE_THETA = 10000.0
WG = 256                          # wo token group
NWG = S // WG                     # 8 per batch

_CACHE = {}


def _consts():
    import ml_dtypes
    bf16 = ml_dtypes.bfloat16
    i = np.arange(HD // 2)
    inv = 1.0 / (ROPE_THETA ** (2 * i / HD))
    t = np.arange(S)
    ang = np.outer(inv, t)  # [64, S]
    cosT = np.repeat(np.cos(ang), 2, axis=0).astype(bf16)  # [128, S]
    sinT = np.repeat(np.sin(ang), 2, axis=0).astype(bf16)
    perm = np.zeros((128, 128), np.float32)
    for j in range(64):
        perm[2 * j, 2 * j + 1] = 1.0
        perm[2 * j + 1, 2 * j] = -1.0
    tri = (np.arange(128)[:, None] <= np.arange(128)[None, :]).astype(np.float32)
    ident = np.eye(128, dtype=np.float32)
    ones = np.ones((128, 1), np.float32)
    ones_row = np.ones((1, 128), np.float32)
    return (cosT, sinT, perm.astype(bf16), tri.astype(bf16),
            ident.astype(bf16), ones.astype(bf16), ones_row.astype(bf16))


def _build():
    import concourse.mybir as mybir
    import concourse.tile as tile
    from concourse import bacc

    F32 = mybir.dt.float32
    BF16 = mybir.dt.bfloat16

    nc = bacc.Bacc("TRN2", target_bir_lowering=False, debug=False,
                   num_devices=N_CORES)

    # bf16 inputs (host-cast during sharding); xs is the FULL x
    xs = nc.dram_tensor("xs", [T, DM], BF16, kind="ExternalInput")
    wq = nc.dram_tensor("wq", [DM, HDQ], BF16, kind="ExternalInput")
    wk = nc.dram_tensor("wk", [DM, HD], BF16, kind="ExternalInput")
    wv = nc.dram_tensor("wv", [DM, HD], BF16, kind="ExternalInput")
    wo = nc.dram_tensor("wo", [DM, HDQ], BF16, kind="ExternalInput")
    cosc = nc.dram_tensor("cosc", [128, S], BF16, kind="ExternalInput")
    sinc = nc.dram_tensor("sinc", [128, S], BF16, kind="ExternalInput")
    permc = nc.dram_tensor("permc", [128, 128], BF16, kind="ExternalInput")
    tric = nc.dram_tensor("tric", [128, 128], BF16, kind="ExternalInput")
    identc = nc.dram_tensor("identc", [128, 128], BF16, kind="ExternalInput")
    onesc = nc.dram_tensor("onesc", [128, 1], BF16, kind="ExternalInput")
    onesrc = nc.dram_tensor("onesrc", [1, 128], BF16, kind="ExternalInput")

    y = nc.dram_tensor("y", [T, HDQ], F32, kind="ExternalOutput")

    rg = [list(range(N_CORES))]

    with tile.TileContext(nc) as tc:
        with (
            tc.tile_pool(name="dram", bufs=1, space="DRAM") as dram,
            tc.tile_pool(name="const", bufs=1) as cp,
        ):
            cos_sb = cp.tile([128, S], BF16, tag="cos")
            sin_sb = cp.tile([128, S], BF16, tag="sin")
            perm_sb = cp.tile([128, 128], BF16, tag="perm")
            tri_sb = cp.tile([128, 128], BF16, tag="tri")
            id_sb = cp.tile([128, 128], BF16, tag="id")
            ones_sb = cp.tile([128, 1], BF16, tag="ones")
            onesr_sb = cp.tile([1, 128], BF16, tag="onesr")
            nc.sync.dma_start(out=cos_sb[:], in_=cosc.ap())
            nc.sync.dma_start(out=sin_sb[:], in_=sinc.ap())
            nc.sync.dma_start(out=perm_sb[:], in_=permc.ap())
            nc.sync.dma_start(out=tri_sb[:], in_=tric.ap())
            nc.sync.dma_start(out=id_sb[:], in_=identc.ap())
            nc.sync.dma_start(out=ones_sb[:], in_=onesc.ap())
            nc.sync.dma_start(out=onesr_sb[:], in_=onesrc.ap())

            t = dict(
                mybir=mybir, F32=F32, BF16=BF16, rg=rg, tc=tc,
                xs=xs, wq=wq, wk=wk, wv=wv, wo=wo, y=y,
                cos_sb=cos_sb, sin_sb=sin_sb, perm_sb=perm_sb,
                tri_sb=tri_sb, id_sb=id_sb, ones_sb=ones_sb,
                onesr_sb=onesr_sb, dram=dram,
            )
            t["oT_h"] = [dram.tile([HDQ, S], BF16, name=f"oT_h{b}")
                         for b in range(B)]
            t["oT_F"] = [dram.tile([DM, S], BF16, addr_space="Shared",
                                   name=f"oT_F{b}") for b in range(B)]
            _emit_body(nc, tc, t)

    nc.compile()
    return nc


def _emit_qkv(nc, tc, t, b):
    """Transpose x (full dim) block by block + QKV projection + RoPE."""
    mybir = t["mybir"]
    F32, BF16 = t["F32"], t["BF16"]
    xs = t["xs"]
    cos_sb, sin_sb = t["cos_sb"], t["sin_sb"]
    perm_sb, id_sb = t["perm_sb"], t["id_sb"]
    qT, kT, v_nat = t["qT"], t["kT"], t["v_nat"]
    wq_sb, wk_sb, wv_sb = t["wq_sb"], t["wk_sb"], t["wv_sb"]

    xtb = t["xtb"]
    with (
        tc.tile_pool(name=f"ps_acc{b}", bufs=1, space="PSUM") as ps_acc,
        tc.tile_pool(name=f"ps_scr{b}", bufs=1, space="PSUM") as ps_scr,
        tc.tile_pool(name=f"wqk{b}", bufs=2) as wp,
    ):
        for tb in range(NTB):
            tsl = slice(tb * TB, (tb + 1) * TB)
            # ---- load this 512-token block of x transposed (DMA xbar) ----
            row = b * S + tb * TB
            xT_blk = xtb.tile([128, KC * TB], BF16, tag="xT_blk")
            for kc in range(KC):
                nc.sync.dma_start_transpose(
                    out=xT_blk[:, kc * TB:(kc + 1) * TB],
                    in_=xs.ap()[row:row + TB, kc * 128:(kc + 1) * 128],
                )

            # ---- QKV projection for this block ----
            psq = [ps_acc.tile([128, TB], F32, tag=f"psq{i}", name=f"psq{i}")
                   for i in range(NQH)]
            psk = ps_acc.tile([128, TB], F32, tag="psk")
            psv = ps_acc.tile([128, TB], F32, tag="psv")
            for kc in range(KC):
                xt_mv = xT_blk[:, kc * TB:(kc + 1) * TB]
                for i in range(NQH):
                    nc.tensor.matmul(
                        psq[i][:],
                        wq_sb[:, kc * HDQ + i * HD:kc * HDQ + (i + 1) * HD],
                        xt_mv,
                        start=(kc == 0), stop=(kc == KC - 1),
                    )
                nc.tensor.matmul(
                    psk[:], wk_sb[:, kc * HD:(kc + 1) * HD], xt_mv,
                    start=(kc == 0), stop=(kc == KC - 1),
                )
                nc.tensor.matmul(
                    psv[:], wv_sb[:, kc * HD:(kc + 1) * HD], xt_mv,
                    start=(kc == 0), stop=(kc == KC - 1),
                )

            # ---- RoPE (q heads + k) ----
            cos_t = cos_sb[:, tsl]
            sin_t = sin_sb[:, tsl]
            for idx in range(NQH + 1):
                acc = psq[idx] if idx < NQH else psk
                dest = qT[idx][:] if idx < NQH else kT[:]
                raw = wp.tile([128, TB], BF16, tag="rope_raw")
                nc.vector.tensor_copy(raw[:], acc[:])
                swp = ps_scr.tile([128, TB], F32, tag="scr")
                nc.tensor.matmul(swp[:], perm_sb[:], raw[:],
                                 start=True, stop=True)
                swp_sb = wp.tile([128, TB], BF16, tag="rope_swp")
                nc.vector.tensor_copy(swp_sb[:], swp[:])
                t1 = wp.tile([128, TB], BF16, tag="rope_t1")
                nc.vector.tensor_mul(t1[:], raw[:], cos_t)
                t2 = wp.tile([128, TB], BF16, tag="rope_t2")
                nc.vector.tensor_mul(t2[:], swp_sb[:], sin_t)
                nc.vector.tensor_add(dest[:, tsl], t1[:], t2[:])

            # ---- V to natural layout ----
            vt_sb = wp.tile([128, TB], BF16, tag="vt_sb")
            nc.vector.tensor_copy(vt_sb[:], psv[:])
            vp = ps_scr.tile([128, TB], BF16, tag="scr")
            for j in range(TB // 128):
                nc.tensor.transpose(
                    vp[:, j * 128:(j + 1) * 128],
                    vt_sb[:, j * 128:(j + 1) * 128], id_sb[:])
            nc.vector.tensor_copy(v_nat[:, tb * TB:(tb + 1) * TB], vp[:])


def _emit_attn(nc, tc, t, b):
    mybir = t["mybir"]
    F32, BF16 = t["F32"], t["BF16"]
    qT, kT, v_nat = t["qT"], t["kT"], t["v_nat"]
    tri_sb, ones_sb, onesr_sb = t["tri_sb"], t["ones_sb"], t["onesr_sb"]
    oT_h = t["oT_h"]

    with (
        tc.tile_pool(name=f"ps_s{b}", bufs=2, space="PSUM") as ps_s,
        tc.tile_pool(name=f"ps_o{b}", bufs=2, space="PSUM") as ps_o,
        tc.tile_pool(name=f"ps_sum{b}", bufs=1, space="PSUM") as ps_sum,
        tc.tile_pool(name=f"ps_rb{b}", bufs=1, space="PSUM") as ps_rb,
        tc.tile_pool(name=f"wa{b}", bufs=2) as wp,
        tc.tile_pool(name=f"ptp{b}", bufs=3) as ptp,
    ):
        for h in range(NQH):
            for qb in range(NQB):
                q0 = qb * QB
                nkt = (q0 + QB) // 128
                kt_max = nkt - 1
                oT = ps_o.tile([128, QB], F32, tag="oT")
                sums = ps_sum.tile([1, QB], F32, tag="sums")
                for pr in range(nkt // 2):
                    kt0, kt1 = 2 * pr, 2 * pr + 1
                    sT = ps_s.tile([128, 2 * QB], F32, tag="sT")
                    pT = ptp.tile([128, 2 * QB], BF16, tag="pT")
                    offs = [max(0, kt * 128 - q0) for kt in (kt0, kt1)]
                    for j, kt in ((0, kt0), (1, kt1)):
                        off = offs[j]
                        nc.tensor.matmul(
                            sT[:, j * QB + off:(j + 1) * QB],
                            kT[:, kt * 128:(kt + 1) * 128],
                            qT[h][:, q0 + off:q0 + QB],
                            start=True, stop=True,
                        )
                    nc.scalar.activation(
                        pT[:, offs[0]:2 * QB],
                        sT[:, offs[0]:2 * QB],
                        mybir.ActivationFunctionType.Exp,
                        scale=SCALE,
                    )
                    for j, kt in ((0, kt0), (1, kt1)):
                        if kt * 128 >= q0:
                            off = j * QB + (kt * 128 - q0)
                            nc.vector.tensor_mul(
                                pT[:, off:off + 128],
                                pT[:, off:off + 128],
                                tri_sb[:],
                            )
                    for j, kt in ((0, kt0), (1, kt1)):
                        off = offs[j]
                        nc.tensor.matmul(
                            oT[:, off:QB],
                            v_nat[:, kt * 128:(kt + 1) * 128],
                            pT[:, j * QB + off:(j + 1) * QB],
                            start=(kt == 0), stop=(kt == kt_max),
                        )
                        nc.tensor.matmul(
                            sums[0:1, off:QB], ones_sb[:],
                            pT[:, j * QB + off:(j + 1) * QB],
                            start=(kt == 0), stop=(kt == kt_max),
                        )
                sums_sb = wp.tile([1, QB], F32, tag="sums_sb")
                nc.vector.tensor_copy(sums_sb[:], sums[0:1, :])
                rec = wp.tile([1, QB], F32, tag="rec")
                scr = wp.tile([1, QB], F32, tag="scr")
                nc.vector.reciprocal_approx_accurate(rec[:], sums_sb[:],
                                                     scr[:])
                rec_bf = wp.tile([1, QB], BF16, tag="rec_bf")
                nc.vector.tensor_copy(rec_bf[:], rec[:])
                rbp = ps_rb.tile([128, QB], F32, tag="rbp")
                nc.tensor.matmul(rbp[:], onesr_sb[:], rec_bf[:],
                                 start=True, stop=True)
                rb_sb = wp.tile([128, QB], F32, tag="rb_sb")
                nc.vector.tensor_copy(rb_sb[:], rbp[:])
                oT_sb = wp.tile([128, QB], BF16, tag="oT_sb")
                nc.vector.tensor_mul(oT_sb[:], oT[:], rb_sb[:])
                # SWDGE store: keeps the Sync HWDGE FIFO free for the next
                # batch's transpose DMAs during attention
                nc.gpsimd.dma_start(
                    out=oT_h[b][:][h * 128:(h + 1) * 128, q0:q0 + QB],
                    in_=oT_sb[:],
                )


def _emit_body(nc, tc, t):
    mybir = t["mybir"]
    F32, BF16 = t["F32"], t["BF16"]
    wq, wk, wv, wo, y = t["wq"], t["wk"], t["wv"], t["wo"], t["y"]
    oT_h, oT_F = t["oT_h"], t["oT_F"]
    rg = t["rg"]

    with (
        tc.tile_pool(name="batch", bufs=1) as bp,
        tc.tile_pool(name="xtb", bufs=2) as xtb,
    ):
        qT = [bp.tile([128, S], BF16, tag=f"qT{h}", name=f"qT{h}")
              for h in range(NQH)]
        kT = bp.tile([128, S], BF16, tag="kT")
        v_nat = bp.tile([128, NKT * 128], BF16, tag="v_nat")
        t["qT"], t["kT"], t["v_nat"] = qT, kT, v_nat
        t["xtb"] = xtb

        with tc.tile_pool(name="wqkv", bufs=1) as wpool:
            wq_sb = wpool.tile([128, KC * HDQ], BF16, tag="wq")
            wk_sb = wpool.tile([128, KC * HD], BF16, tag="wk")
            wv_sb = wpool.tile([128, KC * HD], BF16, tag="wv")
            # scalar HWDGE queue: runs in parallel with the first x
            # transpose DMAs on the Sync queue
            nc.scalar.dma_start(
                out=wq_sb[:].rearrange("p (kc d) -> p kc d", kc=KC),
                in_=wq.ap().rearrange("(kc p) d -> p kc d", p=128),
            )
            nc.scalar.dma_start(
                out=wk_sb[:].rearrange("p (kc d) -> p kc d", kc=KC),
                in_=wk.ap().rearrange("(kc p) d -> p kc d", p=128),
            )
            nc.scalar.dma_start(
                out=wv_sb[:].rearrange("p (kc d) -> p kc d", kc=KC),
                in_=wv.ap().rearrange("(kc p) d -> p kc d", p=128),
            )
            t["wq_sb"], t["wk_sb"], t["wv_sb"] = wq_sb, wk_sb, wv_sb

            _emit_qkv(nc, tc, t, 0)
            _emit_attn(nc, tc, t, 0)
            nc.gpsimd.collective_compute(
                "AllGather", mybir.AluOpType.bypass, replica_groups=rg,
                ins=[oT_h[0][:].opt()], outs=[oT_F[0][:].opt()],
            )
            _emit_qkv(nc, tc, t, 1)

        # ---------- WO projection (column shard) ----------
        with (
            tc.tile_pool(name="wo_p", bufs=1) as wo_p,
            tc.tile_pool(name="otp", bufs=3) as otp,
        ):
            wo_sb = wo_p.tile([128, KC * HDQ], BF16, tag="wo")
            nc.sync.dma_start(
                out=wo_sb[:].rearrange("p (kc d) -> p kc d", kc=KC),
                in_=wo.ap().rearrange("(kc p) d -> p kc d", p=128),
            )

            def strip_load(b, tg, eng=None):
                ot_strip = otp.tile([128, KC * WG], BF16, tag="ot_strip",
                                    name=f"ot_strip{b}_{tg}")
                (eng or nc.scalar).dma_start(
                    out=ot_strip[:].rearrange("p (hc t) -> p hc t", hc=KC),
                    in_=oT_F[b][:]
                    .rearrange("(hc p) t -> p hc t", p=128)
                    [:, :, tg * WG:(tg + 1) * WG],
                )
                return ot_strip

            # prefetch first strips of batch 0 while attn(b1) runs; Sync is
            # idle there and oag0 completes during qkv1, so no head-of-line
            # blocking on the Sync FIFO
            strips0 = [strip_load(0, tg, eng=nc.sync) for tg in range(3)]

            _emit_attn(nc, tc, t, 1)
            nc.gpsimd.collective_compute(
                "AllGather", mybir.AluOpType.bypass, replica_groups=rg,
                ins=[oT_h[1][:].opt()], outs=[oT_F[1][:].opt()],
            )

            with (
                tc.tile_pool(name="ps_y", bufs=3, space="PSUM") as ps_y,
                tc.tile_pool(name="w_wo", bufs=3) as wp,
            ):
                for b in range(B):
                    for tg in range(NWG):
                        if b == 0 and tg < 3:
                            ot_strip = strips0[tg]
                        else:
                            ot_strip = strip_load(b, tg)
                        for ts in range(WG // 128):
                            psy = ps_y.tile([128, HDQ], F32, tag="psy")
                            for hc in range(KC):
                                nc.tensor.matmul(
                                    psy[:],
                                    ot_strip[:, hc * WG + ts * 128:
                                             hc * WG + (ts + 1) * 128],
                                    wo_sb[:, hc * HDQ:(hc + 1) * HDQ],
                                    start=(hc == 0), stop=(hc == KC - 1),
                                )
                            y_sb = wp.tile([128, HDQ], F32, tag="y_sb")
                            nc.vector.tensor_copy(y_sb[:], psy[:])
                            row = b * S + tg * WG + ts * 128
                            nc.sync.dma_start(out=y.ap()[row:row + 128, :],
                                              in_=y_sb[:])


def _in_maps(x, wq, wk, wv, wo):
    import ml_dtypes
    bf16 = ml_dtypes.bfloat16
    x_bf = np.ascontiguousarray(
        np.asarray(x, dtype=np.float32).reshape(T, DM)).astype(bf16)
    cosT, sinT, perm, tri, ident, ones, ones_row = _consts()
    wq = np.asarray(wq, np.float32)
    wk = np.asarray(wk, np.float32)
    wv = np.asarray(wv, np.float32)
    wo = np.asarray(wo, np.float32)
    maps = []
    for c in range(N_CORES):
        qsl = slice(c * HDQ, (c + 1) * HDQ)
        ksl = slice(c * HD, (c + 1) * HD)
        maps.append({
            "xs": x_bf,
            "wq": np.ascontiguousarray(wq[:, qsl]).astype(bf16),
            "wk": np.ascontiguousarray(wk[:, ksl]).astype(bf16),
            "wv": np.ascontiguousarray(wv[:, ksl]).astype(bf16),
            "wo": np.ascontiguousarray(wo[:, qsl]).astype(bf16),
            "cosc": cosT, "sinc": sinT, "permc": perm, "tric": tri,
            "identc": ident, "onesc": ones, "onesrc": ones_row,
        })
    return maps


def kernel(x, wq, wk, wv, wo, start_pos=0, **_unused):
    from concourse import bass_utils

    assert int(np.asarray(start_pos)) == 0
    in_maps = _in_maps(x, wq, wk, wv, wo)

    if "nc" not in _CACHE:
        _CACHE["nc"] = _build()
    nc = _CACHE["nc"]

    res = bass_utils.run_bass_kernel_spmd(
        nc, in_maps, core_ids=list(range(N_CORES)),
        trace=bool(int(os.environ.get("KERNEL_TRACE", "0") or 0)),
    )
    _CACHE["last_result"] = res

    out = np.empty((T, DM), np.float32)
    for c in range(N_CORES):
        out[:, c * HDQ:(c + 1) * HDQ] = res.results[c]["y"]
    return out.reshape(B, S, DM)


# revision 13
# speedup vs baseline: 1.0149x; 1.0149x over previous
"""Trainium2 Bass kernel for nn_Attention_33354716021131.

Dense GQA attention block (B=2, S=2048, D=4096, 32 q-heads / 8 kv-heads,
head_dim 128, RoPE, causal softmax) tensor-parallel across 8 NeuronCores.

Sharding (per core c):
  - heads: q-heads 4c..4c+3 (one kv-head group c) -> wq/wk/wv column shards
  - x^T computed fully locally: every core PE-transposes the whole x into
    SBUF-resident blocks (no x collective at all -> no exposure to the
    collectives entry barrier / launch skew at kernel start)
  - attention entirely local to the core (its 4 q-heads x 2 batches)
  - attention outputs (head-major, transposed) AllGather -> full O^T, then
    wo column shard: core c computes y[:, 512c:512c+512]; host concatenates.

All matmul operands are bfloat16 (fp32 PSUM accumulation): bf16 streams at
1 cycle/row on the PE (fp32/fp32r modes run ~3x slower and do not engage
the HAM clock un-throttle), and halves every DMA/collective payload.
Inputs are cast to bf16 on the host as part of sharding.
"""
import math
import os

import numpy as np

N_CORES = 8
B = 2
S = 2048
DM = 4096
N_HEADS = 32
HD = 128
NQH = N_HEADS // N_CORES          # 4 q heads per core
HDQ = NQH * HD                    # 512
T = B * S                         # 4096 tokens
KC = DM // 128                    # 32 k-chunks
TB = 512                          # token block for projections
NTB = S // TB                     # 4 per batch
QB = 512                          # query block for attention
NQB = S // QB                     # 4
NKT = S // 128                    # 16 key tiles per batch
SCALE = 1.0 / math.sqrt(HD)
ROPE_THETA = 10000.0
WG = 256                          # wo token group
NWG = S // WG                     # 8 per batch

_CACHE = {}


def _consts():
    import ml_dtypes
    bf16 = ml_dtypes.bfloat16
    i = np.arange(HD // 2)
    inv = 1.0 / (ROPE_THETA ** (2 * i / HD))
    t = np.arange(S)
    ang = np.outer(inv, t)  # [64, S]
    cosT = np.repeat(np.cos(ang), 2, axis=0).astype(bf16)  # [128, S]
    sinT = np.repeat(np.sin(ang), 2, axis=0).astype(bf16)
    perm = np.zeros((128, 128), np.float32)
    for j in range(64):
        perm[2 * j, 2 * j + 1] = 1.0
        perm[2 * j + 1, 2 * j] = -1.0
    tri = (np.arange(128)[:, None] <= np.arange(128)[None, :]).astype(np.float32)
    ident = np.eye(128, dtype=np.float32)
    ones = np.ones((128, 1), np.float32)
    ones_row = np.ones((1, 128), np.float32)
    return (cosT, sinT, perm.astype(bf16), tri.astype(bf16),
            ident.astype(bf16), ones.astype(bf16), ones_row.astype(bf16))


def _build():
    import concourse.mybir as mybir
    import concourse.tile as tile
    from concourse import bacc

    F32 = mybir.dt.float32
    BF16 = mybir.dt.bfloat16

    nc = bacc.Bacc("TRN2", target_bir_lowering=False, debug=False,
                   num_devices=N_CORES)

    # bf16 inputs (host-cast during sharding); xs is the FULL x
    xs = nc.dram_tensor("xs", [T, DM], BF16, kind="ExternalInput")
    wq = nc.dram_tensor("wq", [DM, HDQ], BF16, kind="ExternalInput")
    wk = nc.dram_tensor("wk", [DM, HD], BF16, kind="ExternalInput")
    wv = nc.dram_tensor("wv", [DM, HD], BF16, kind="ExternalInput")
    wo = nc.dram_tensor("wo", [DM, HDQ], BF16, kind="ExternalInput")
    cosc = nc.dram_tensor("cosc", [128, S], BF16, kind="ExternalInput")
    sinc = nc.dram_tensor("sinc", [128, S], BF16, kind="ExternalInput")
    permc = nc.dram_tensor("permc", [128, 128], BF16, kind="ExternalInput")
    tric = nc.dram_tensor("tric", [128, 128], BF16, kind="ExternalInput")
    identc = nc.dram_tensor("identc", [128, 128], BF16, kind="ExternalInput")
    onesc = nc.dram_tensor("onesc", [128, 1], BF16, kind="ExternalInput")
    onesrc = nc.dram_tensor("onesrc", [1, 128], BF16, kind="ExternalInput")

    y = nc.dram_tensor("y", [T, HDQ], F32, kind="ExternalOutput")

    rg = [list(range(N_CORES))]

    with tile.TileContext(nc) as tc:
        with (
            tc.tile_pool(name="dram", bufs=1, space="DRAM") as dram,
            tc.tile_pool(name="const", bufs=1) as cp,
        ):
            cos_sb = cp.tile([128, S], BF16, tag="cos")
            sin_sb = cp.tile([128, S], BF16, tag="sin")
            perm_sb = cp.tile([128, 128], BF16, tag="perm")
            tri_sb = cp.tile([128, 128], BF16, tag="tri")
            id_sb = cp.tile([128, 128], BF16, tag="id")
            ones_sb = cp.tile([128, 1], BF16, tag="ones")
            onesr_sb = cp.tile([1, 128], BF16, tag="onesr")
            nc.sync.dma_start(out=cos_sb[:], in_=cosc.ap())
            nc.sync.dma_start(out=sin_sb[:], in_=sinc.ap())
            nc.sync.dma_start(out=perm_sb[:], in_=permc.ap())
            nc.sync.dma_start(out=tri_sb[:], in_=tric.ap())
            nc.sync.dma_start(out=id_sb[:], in_=identc.ap())
            nc.sync.dma_start(out=ones_sb[:], in_=onesc.ap())
            nc.sync.dma_start(out=onesr_sb[:], in_=onesrc.ap())

            t = dict(
                mybir=mybir, F32=F32, BF16=BF16, rg=rg, tc=tc,
                xs=xs, wq=wq, wk=wk, wv=wv, wo=wo, y=y,
                cos_sb=cos_sb, sin_sb=sin_sb, perm_sb=perm_sb,
                tri_sb=tri_sb, id_sb=id_sb, ones_sb=ones_sb,
                onesr_sb=onesr_sb, dram=dram,
            )
            t["oT_h"] = [dram.tile([HDQ, S], BF16, name=f"oT_h{b}")
                         for b in range(B)]
            t["oT_F"] = [dram.tile([DM, S], BF16, addr_space="Shared",
                                   name=f"oT_F{b}") for b in range(B)]
            _emit_body(nc, tc, t)

    nc.compile()
    return nc


def _emit_qkv(nc, tc, t, b):
    """Transpose x (full dim) block by block + QKV projection + RoPE."""
    mybir = t["mybir"]
    F32, BF16 = t["F32"], t["BF16"]
    xs = t["xs"]
    cos_sb, sin_sb = t["cos_sb"], t["sin_sb"]
    perm_sb, id_sb = t["perm_sb"], t["id_sb"]
    qT, kT, v_nat = t["qT"], t["kT"], t["v_nat"]
    wq_sb, wk_sb, wv_sb = t["wq_sb"], t["wk_sb"], t["wv_sb"]

    xtb = t["xtb"]
    with (
        tc.tile_pool(name=f"ps_acc{b}", bufs=1, space="PSUM") as ps_acc,
        tc.tile_pool(name=f"ps_scr{b}", bufs=1, space="PSUM") as ps_scr,
        tc.tile_pool(name=f"wqk{b}", bufs=2) as wp,
    ):
        for tb in range(NTB):
            tsl = slice(tb * TB, (tb + 1) * TB)
            # ---- load this 512-token block of x transposed (DMA xbar) ----
            row = b * S + tb * TB
            xT_blk = xtb.tile([128, KC * TB], BF16, tag="xT_blk")
            for kc in range(KC):
                nc.sync.dma_start_transpose(
                    out=xT_blk[:, kc * TB:(kc + 1) * TB],
                    in_=xs.ap()[row:row + TB, kc * 128:(kc + 1) * 128],
                )

            # ---- QKV projection for this block ----
            psq = [ps_acc.tile([128, TB], F32, tag=f"psq{i}", name=f"psq{i}")
                   for i in range(NQH)]
            psk = ps_acc.tile([128, TB], F32, tag="psk")
            psv = ps_acc.tile([128, TB], F32, tag="psv")
            for kc in range(KC):
                xt_mv = xT_blk[:, kc * TB:(kc + 1) * TB]
                for i in range(NQH):
                    nc.tensor.matmul(
                        psq[i][:],
                        wq_sb[:, kc * HDQ + i * HD:kc * HDQ + (i + 1) * HD],
                        xt_mv,
                        start=(kc == 0), stop=(kc == KC - 1),
                    )
                nc.tensor.matmul(
                    psk[:], wk_sb[:, kc * HD:(kc + 1) * HD], xt_mv,
                    start=(kc == 0), stop=(kc == KC - 1),
                )
                nc.tensor.matmul(
                    psv[:], wv_sb[:, kc * HD:(kc + 1) * HD], xt_mv,
                    start=(kc == 0), stop=(kc == KC - 1),
                )

            # ---- RoPE (q heads + k) ----
            cos_t = cos_sb[:, tsl]
            sin_t = sin_sb[:, tsl]
            for idx in range(NQH + 1):
                acc = psq[idx] if idx < NQH else psk
                dest = qT[idx][:] if idx < NQH else kT[:]
                raw = wp.tile([128, TB], BF16, tag="rope_raw")
                nc.vector.tensor_copy(raw[:], acc[:])
                swp = ps_scr.tile([128, TB], F32, tag="scr")
                nc.tensor.matmul(swp[:], perm_sb[:], raw[:],
                                 start=True, stop=True)
                swp_sb = wp.tile([128, TB], BF16, tag="rope_swp")
                nc.vector.tensor_copy(swp_sb[:], swp[:])
                t1 = wp.tile([128, TB], BF16, tag="rope_t1")
                nc.vector.tensor_mul(t1[:], raw[:], cos_t)
                t2 = wp.tile([128, TB], BF16, tag="rope_t2")
                nc.vector.tensor_mul(t2[:], swp_sb[:], sin_t)
                nc.vector.tensor_add(dest[:, tsl], t1[:], t2[:])

            # ---- V to natural layout ----
            vt_sb = wp.tile([128, TB], BF16, tag="vt_sb")
            nc.vector.tensor_copy(vt_sb[:], psv[:])
            vp = ps_scr.tile([128, TB], BF16, tag="scr")
            for j in range(TB // 128):
                nc.tensor.transpose(
                    vp[:, j * 128:(j + 1) * 128],
                    vt_sb[:, j * 128:(j + 1) * 128], id_sb[:])
            nc.vector.tensor_copy(v_nat[:, tb * TB:(tb + 1) * TB], vp[:])


def _emit_attn(nc, tc, t, b):
    mybir = t["mybir"]
    F32, BF16 = t["F32"], t["BF16"]
    qT, kT, v_nat = t["qT"], t["kT"], t["v_nat"]
    tri_sb, ones_sb, onesr_sb = t["tri_sb"], t["ones_sb"], t["onesr_sb"]
    oT_h = t["oT_h"]

    with (
        tc.tile_pool(name=f"ps_s{b}", bufs=2, space="PSUM") as ps_s,
        tc.tile_pool(name=f"ps_o{b}", bufs=2, space="PSUM") as ps_o,
        tc.tile_pool(name=f"ps_sum{b}", bufs=1, space="PSUM") as ps_sum,
        tc.tile_pool(name=f"ps_rb{b}", bufs=1, space="PSUM") as ps_rb,
        tc.tile_pool(name=f"wa{b}", bufs=2) as wp,
        tc.tile_pool(name=f"ptp{b}", bufs=4) as ptp,
    ):
        for h in range(NQH):
            for qb in range(NQB):
                q0 = qb * QB
                nkt = (q0 + QB) // 128
                kt_max = nkt - 1
                oT = ps_o.tile([128, QB], F32, tag="oT")
                sums = ps_sum.tile([1, QB], F32, tag="sums")
                for pr in range(nkt // 2):
                    kt0, kt1 = 2 * pr, 2 * pr + 1
                    sT = ps_s.tile([128, 2 * QB], F32, tag="sT")
                    pT = ptp.tile([128, 2 * QB], BF16, tag="pT")
                    offs = [max(0, kt * 128 - q0) for kt in (kt0, kt1)]
                    for j, kt in ((0, kt0), (1, kt1)):
                        off = offs[j]
                        nc.tensor.matmul(
                            sT[:, j * QB + off:(j + 1) * QB],
                            kT[:, kt * 128:(kt + 1) * 128],
                            qT[h][:, q0 + off:q0 + QB],
                            start=True, stop=True,
                        )
                    nc.scalar.activation(
                        pT[:, offs[0]:2 * QB],
                        sT[:, offs[0]:2 * QB],
                        mybir.ActivationFunctionType.Exp,
                        scale=SCALE,
                    )
                    for j, kt in ((0, kt0), (1, kt1)):
                        if kt * 128 >= q0:
                            off = j * QB + (kt * 128 - q0)
                            nc.vector.tensor_mul(
                                pT[:, off:off + 128],
                                pT[:, off:off + 128],
                                tri_sb[:],
                            )
                    for j, kt in ((0, kt0), (1, kt1)):
                        off = offs[j]
                        nc.tensor.matmul(
                            oT[:, off:QB],
                            v_nat[:, kt * 128:(kt + 1) * 128],
                            pT[:, j * QB + off:(j + 1) * QB],
                            start=(kt == 0), stop=(kt == kt_max),
                        )
                        nc.tensor.matmul(
                            sums[0:1, off:QB], ones_sb[:],
                            pT[:, j * QB + off:(j + 1) * QB],
                            start=(kt == 0), stop=(kt == kt_max),
                        )
                sums_sb = wp.tile([1, QB], F32, tag="sums_sb")
                nc.vector.tensor_copy(sums_sb[:], sums[0:1, :])
                rec = wp.tile([1, QB], F32, tag="rec")
                scr = wp.tile([1, QB], F32, tag="scr")
                nc.vector.reciprocal_approx_accurate(rec[:], sums_sb[:],
                                                     scr[:])
                rec_bf = wp.tile([1, QB], BF16, tag="rec_bf")
                nc.vector.tensor_copy(rec_bf[:], rec[:])
                rbp = ps_rb.tile([128, QB], F32, tag="rbp")
                nc.tensor.matmul(rbp[:], onesr_sb[:], rec_bf[:],
                                 start=True, stop=True)
                rb_sb = wp.tile([128, QB], F32, tag="rb_sb")
                nc.vector.tensor_copy(rb_sb[:], rbp[:])
                oT_sb = wp.tile([128, QB], BF16, tag="oT_sb")
                nc.vector.tensor_mul(oT_sb[:], oT[:], rb_sb[:])
                # SWDGE store: keeps the Sync HWDGE FIFO free for the next
                # batch's transpose DMAs during attention
                nc.gpsimd.dma_start(
                    out=oT_h[b][:][h * 128:(h + 1) * 128, q0:q0 + QB],
                    in_=oT_sb[:],
                )


def _emit_body(nc, tc, t):
    mybir = t["mybir"]
    F32, BF16 = t["F32"], t["BF16"]
    wq, wk, wv, wo, y = t["wq"], t["wk"], t["wv"], t["wo"], t["y"]
    oT_h, oT_F = t["oT_h"], t["oT_F"]
    rg = t["rg"]

    with (
        tc.tile_pool(name="batch", bufs=1) as bp,
        tc.tile_pool(name="xtb", bufs=2) as xtb,
    ):
        qT = [bp.tile([128, S], BF16, tag=f"qT{h}", name=f"qT{h}")
              for h in range(NQH)]
        kT = bp.tile([128, S], BF16, tag="kT")
        v_nat = bp.tile([128, NKT * 128], BF16, tag="v_nat")
        t["qT"], t["kT"], t["v_nat"] = qT, kT, v_nat
        t["xtb"] = xtb

        with tc.tile_pool(name="wqkv", bufs=1) as wpool:
            wq_sb = wpool.tile([128, KC * HDQ], BF16, tag="wq")
            wk_sb = wpool.tile([128, KC * HD], BF16, tag="wk")
            wv_sb = wpool.tile([128, KC * HD], BF16, tag="wv")
            # scalar HWDGE queue: runs in parallel with the first x
            # transpose DMAs on the Sync queue
            nc.scalar.dma_start(
                out=wq_sb[:].rearrange("p (kc d) -> p kc d", kc=KC),
                in_=wq.ap().rearrange("(kc p) d -> p kc d", p=128),
            )
            nc.scalar.dma_start(
                out=wk_sb[:].rearrange("p (kc d) -> p kc d", kc=KC),
                in_=wk.ap().rearrange("(kc p) d -> p kc d", p=128),
            )
            nc.scalar.dma_start(
                out=wv_sb[:].rearrange("p (kc d) -> p kc d", kc=KC),
                in_=wv.ap().rearrange("(kc p) d -> p kc d", p=128),
            )
            t["wq_sb"], t["wk_sb"], t["wv_sb"] = wq_sb, wk_sb, wv_sb

            _emit_qkv(nc, tc, t, 0)
            _emit_attn(nc, tc, t, 0)
            nc.gpsimd.collective_compute(
                "AllGather", mybir.AluOpType.bypass, replica_groups=rg,
                ins=[oT_h[0][:].opt()], outs=[oT_F[0][:].opt()],
            )
            _emit_qkv(nc, tc, t, 1)

        # ---------- WO projection (column shard) ----------
        with (
            tc.tile_pool(name="wo_p", bufs=1) as wo_p,
            tc.tile_pool(name="otp", bufs=3) as otp,
        ):
            wo_sb = wo_p.tile([128, KC * HDQ], BF16, tag="wo")
            nc.sync.dma_start(
                out=wo_sb[:].rearrange("p (kc d) -> p kc d", kc=KC),
                in_=wo.ap().rearrange("(kc p) d -> p kc d", p=128),
            )

            def strip_load(b, tg, eng=None):
                ot_strip = otp.tile([128, KC * WG], BF16, tag="ot_strip",
                                    name=f"ot_strip{b}_{tg}")
                (eng or nc.scalar).dma_start(
                    out=ot_strip[:].rearrange("p (hc t) -> p hc t", hc=KC),
                    in_=oT_F[b][:]
                    .rearrange("(hc p) t -> p hc t", p=128)
                    [:, :, tg * WG:(tg + 1) * WG],
                )
                return ot_strip

            # prefetch first strips of batch 0 while attn(b1) runs; Sync is
            # idle there and oag0 completes during qkv1, so no head-of-line
            # blocking on the Sync FIFO
            strips0 = [strip_load(0, tg, eng=nc.sync) for tg in range(3)]

            _emit_attn(nc, tc, t, 1)
            nc.gpsimd.collective_compute(
                "AllGather", mybir.AluOpType.bypass, replica_groups=rg,
                ins=[oT_h[1][:].opt()], outs=[oT_F[1][:].opt()],
            )

            with (
                tc.tile_pool(name="ps_y", bufs=4, space="PSUM") as ps_y,
                tc.tile_pool(name="w_wo", bufs=3) as wp,
            ):
                for b in range(B):
                    for tg in range(NWG):
                        if b == 0 and tg < 3:
                            ot_strip = strips0[tg]
                        else:
                            ot_strip = strip_load(b, tg)
                        for ts in range(WG // 128):
                            psy = ps_y.tile([128, HDQ], F32, tag="psy")
                            for hc in range(KC):
                                nc.tensor.matmul(
                                    psy[:],
                                    ot_strip[:, hc * WG + ts * 128:
                                             hc * WG + (ts + 1) * 128],
                                    wo_sb[:, hc * HDQ:(hc + 1) * HDQ],
                                    start=(hc == 0), stop=(hc == KC - 1),
                                )
                            y_sb = wp.tile([128, HDQ], F32, tag="y_sb")
                            nc.vector.tensor_copy(y_sb[:], psy[:])
                            row = b * S + tg * WG + ts * 128
                            nc.sync.dma_start(out=y.ap()[row:row + 128, :],
                                              in_=y_sb[:])


def _in_maps(x, wq, wk, wv, wo):
    import ml_dtypes
    bf16 = ml_dtypes.bfloat16
    x_bf = np.ascontiguousarray(
        np.asarray(x, dtype=np.float32).reshape(T, DM)).astype(bf16)
    cosT, sinT, perm, tri, ident, ones, ones_row = _consts()
    wq = np.asarray(wq, np.float32)
    wk = np.asarray(wk, np.float32)
    wv = np.asarray(wv, np.float32)
    wo = np.asarray(wo, np.float32)
    maps = []
    for c in range(N_CORES):
        qsl = slice(c * HDQ, (c + 1) * HDQ)
        ksl = slice(c * HD, (c + 1) * HD)
        maps.append({
            "xs": x_bf,
            "wq": np.ascontiguousarray(wq[:, qsl]).astype(bf16),
            "wk": np.ascontiguousarray(wk[:, ksl]).astype(bf16),
            "wv": np.ascontiguousarray(wv[:, ksl]).astype(bf16),
            "wo": np.ascontiguousarray(wo[:, qsl]).astype(bf16),
            "cosc": cosT, "sinc": sinT, "permc": perm, "tric": tri,
            "identc": ident, "onesc": ones, "onesrc": ones_row,
        })
    return maps


def kernel(x, wq, wk, wv, wo, start_pos=0, **_unused):
    from concourse import bass_utils

    assert int(np.asarray(start_pos)) == 0
    in_maps = _in_maps(x, wq, wk, wv, wo)

    if "nc" not in _CACHE:
        _CACHE["nc"] = _build()
    nc = _CACHE["nc"]

    res = bass_utils.run_bass_kernel_spmd(
        nc, in_maps, core_ids=list(range(N_CORES)),
        trace=bool(int(os.environ.get("KERNEL_TRACE", "0") or 0)),
    )
    _CACHE["last_result"] = res

    out = np.empty((T, DM), np.float32)
    for c in range(N_CORES):
        out[:, c * HDQ:(c + 1) * HDQ] = res.results[c]["y"]
    return out.reshape(B, S, DM)
